# revision 11
# baseline (speedup 1.0000x reference)
"""Trainium2 Bass kernel for nn_DecoderRNN (GRU decoder, 140 sequential steps).

Strategy (data-parallel, per sharding hint):
  - B=512 sharded 8 ways -> 64 batch rows per core; weights replicated.
  - Feature-major on-chip layout: a [F, B] tensor is F/128 chunks of
    [128 partitions, 64 batch] side by side in the free dim.
  - Weight fusion: the fed-back x = h @ out_W.T + out_b is linear in h, so
    it is folded into the next step's embedding (M1 = out_W.T @ emb_W.T)
    and into the decoder output projection (Wyd = out_W.T @ reg_W.T).
    x is never materialized; one full serial stage per step disappears.
  - Gate math runs as fused [128, 256] ops (1 relu, 2 sigmoid, 1 tanh on
    the scalar engine; 5 tensor_tensor on DVE) instead of per-chunk
    [128, 64] ops - per-op fixed overhead dominates at this size.
  - All gate biases are folded into PSUM by tiny K=1 matmuls (lhsT = bias
    row [1,128], rhs = ones [1,64]) at the start of each accumulation
    group, so the fused activations need no per-chunk bias APs.
  - PE phase order per step keeps independent matmul work queued behind
    every cross-engine dependency so the PE (and its DVFS clock) stays fed.
  - Everything bf16 except PSUM (fp32), tanh pre-activation (fp32) and the
    f32 output; validated vs f32 reference at rel err ~2e-3.
"""

import numpy as np
import ml_dtypes

B, T_ENC, E, H, O, PRED_LEN = 512, 140, 256, 512, 64, 140
NCORES = 8
BC = B // NCORES           # 64 batch rows per core
T_ALL = T_ENC + PRED_LEN   # 280

bf16 = ml_dtypes.bfloat16


def _pack_tiles(wT, n_k, n_m):
    """Pack a [K, M] (pre-transposed) weight into [128, n_m*n_k*128] bf16:
    tile (mi, k) at cols (mi*n_k + k)*128."""
    K, M = wT.shape
    assert K == n_k * 128 and M == n_m * 128
    t = wT.reshape(n_k, 128, n_m, 128).transpose(2, 0, 1, 3)  # [mc, kc, 128, 128]
    t = t.transpose(2, 0, 1, 3).reshape(128, -1)
    return np.ascontiguousarray(t.astype(bf16))


def _feat_major(x, n_chunks):
    """[B, F] -> [128, n_chunks*B] feature-major chunk layout."""
    b, f = x.shape
    assert f == n_chunks * 128
    t = x.reshape(b, n_chunks, 128).transpose(2, 1, 0).reshape(128, n_chunks * b)
    return np.ascontiguousarray(t)


def build_program(nsteps=PRED_LEN, t_enc=T_ENC, lowering=True):
    """Build the Bass program (per-core SPMD). Returns nc."""
    import concourse.bass as bass
    import concourse.tile as tile
    from concourse import bacc, mybir

    AF = mybir.ActivationFunctionType
    OP = mybir.AluOpType
    f32 = mybir.dt.float32
    bf = mybir.dt.bfloat16

    t_all = t_enc + nsteps

    if lowering:
        nc = bacc.Bacc("TRN2", target_bir_lowering=True, debug=False)
    else:
        nc = bass.Bass("TRN2", target_bir_lowering=False, debug=False)

    # ---- DRAM I/O ----
    encT_d = nc.dram_tensor("encT", [128, t_enc * 128], bf, kind="ExternalInput").ap()
    h0_d = nc.dram_tensor("h0T", [128, 4 * BC], bf, kind="ExternalInput").ap()
    m1_d = nc.dram_tensor("m1T", [128, 16 * 128], bf, kind="ExternalInput").ap()
    wih_d = nc.dram_tensor("wihT", [128, 48 * 128], bf, kind="ExternalInput").ap()
    whh_d = nc.dram_tensor("whhT", [128, 48 * 128], bf, kind="ExternalInput").ap()
    emb_d = nc.dram_tensor("embT", [128, 8 * 128], bf, kind="ExternalInput").ap()
    wyd_d = nc.dram_tensor("wydT", [128, 4 * O], bf, kind="ExternalInput").ap()
    regw_d = nc.dram_tensor("regwT", [128, 2 * O], bf, kind="ExternalInput").ap()
    # bias lhsT rows, all on partition 0: [1, n*128] order:
    # e(4) e0(4) r(4) z(4) hn(4) inn(4)  (each 4 rows of 128)
    bias_d = nc.dram_tensor("biasP", [1, 24 * 128], bf, kind="ExternalInput").ap()
    # ones (stream for gate-bias mm is its lhsT; rhs rows for y biases)
    ones_d = nc.dram_tensor("ones1", [1, BC], bf, kind="ExternalInput").ap()
    ybias_d = nc.dram_tensor("ybias", [1, 2 * O], bf, kind="ExternalInput").ap()
    y_d = nc.dram_tensor("y", [BC, t_all, O], f32, kind="ExternalOutput").ap()

    with tile.TileContext(nc) as tc:
        import contextlib
        with contextlib.ExitStack() as ctx:
            consts = ctx.enter_context(tc.tile_pool(name="consts", bufs=1))
            temps = ctx.enter_context(tc.tile_pool(name="temps", bufs=2))
            ytmp = ctx.enter_context(tc.tile_pool(name="ytmp", bufs=3))
            psum = ctx.enter_context(tc.tile_pool(name="psum", bufs=1, space="PSUM"))

            # ---- ACT table warmup (see baseline note: pin the table load
            # to dependency-light dummy ops) ----
            wt = consts.tile([128, 10], f32, tag="wtbl", name="wtbl")
            nc.vector.memset(wt[:, 0:5], 0.0)
            nc.scalar.activation(wt[:, 5:6], wt[:, 0:1], AF.Relu)
            nc.scalar.activation(wt[:, 6:7], wt[:, 1:2], AF.Sigmoid)
            nc.scalar.activation(wt[:, 7:8], wt[:, 2:3], AF.Tanh)
            nc.scalar.activation(wt[:, 8:9], wt[:, 3:4], AF.Identity)

            # ---- load constants into SBUF ----
            m1_sb = consts.tile([128, 16 * 128], bf, tag="m1")
            wih_sb = consts.tile([128, 48 * 128], bf, tag="wih")
            whh_sb = consts.tile([128, 48 * 128], bf, tag="whh")
            emb_sb = consts.tile([128, 8 * 128], bf, tag="emb")
            wyd_sb = consts.tile([128, 4 * O], bf, tag="wyd")
            regw_sb = consts.tile([128, 2 * O], bf, tag="regw")
            bias_sb = consts.tile([1, 24 * 128], bf, tag="biasP")
            ones_sb = consts.tile([1, BC], bf, tag="ones1")
            ybias_sb = consts.tile([1, 2 * O], bf, tag="ybias")
            encT_sb = consts.tile([128, t_enc * 128], bf, tag="encT")

            nc.sync.dma_start(out=emb_sb, in_=emb_d)
            nc.sync.dma_start(out=m1_sb, in_=m1_d)
            nc.sync.dma_start(out=whh_sb, in_=whh_d)
            nc.sync.dma_start(out=wih_sb, in_=wih_d)
            nc.sync.dma_start(out=wyd_sb, in_=wyd_d)
            nc.sync.dma_start(out=regw_sb, in_=regw_d)
            nc.sync.dma_start(out=bias_sb, in_=bias_d)
            nc.sync.dma_start(out=ones_sb, in_=ones_d)
            nc.sync.dma_start(out=ybias_sb, in_=ybias_d)
            # x0 block (last encoder token) first so step 0 can start early
            lastblk = slice((t_enc - 1) * 128, t_enc * 128)
            nc.sync.dma_start(out=encT_sb[:, lastblk], in_=encT_d[:, lastblk])
            nsplit = 4
            per = (t_enc - 1) // nsplit + 1
            for i in range(nsplit):
                lo, hi = i * per, min((i + 1) * per, t_enc - 1)
                if lo >= hi:
                    continue
                nc.sync.dma_start(out=encT_sb[:, lo * 128:hi * 128],
                                  in_=encT_d[:, lo * 128:hi * 128])

            # ---- persistent state ----
            h_sb = consts.tile([128, 4 * BC], bf, tag="h", name="h")
            nc.sync.dma_start(out=h_sb, in_=h0_d)

            # ---- persistent PSUM regions (one bank each: a matmul with
            # start=True arms a whole 2KB zero region, and only one
            # accumulation group may be open per bank) ----
            e_ps = psum.tile([128, 512], f32, tag="eps")       # e pre-act
            r_ps = psum.tile([128, 512], f32, tag="rps")       # a_r
            z_ps = psum.tile([128, 512], f32, tag="zps")       # a_z
            hn_ps = psum.tile([128, 512], f32, tag="hnps")     # hn
            in_ps = psum.tile([128, 512], f32, tag="inps")     # inn
            y_ps = psum.tile([BC, 512], f32, tag="yps")        # yenc | ydec

            def wtile(sb, mi, k, n_k):
                j = (mi * n_k + k) * 128
                return sb[:, j:j + 128]

            def brow(idx):
                return bias_sb[0:1, idx * 128:(idx + 1) * 128]

            # bias row base indices in bias_sb
            BE, BE0, BR, BZ, BHN, BIN = 0, 4, 8, 12, 16, 20

            def gate_group(dst, bias_base, w_sb, m_global, src, n_k,
                           first=False, last=False):
                """m-quarter of a bank-level accumulation group. `first` puts
                start=True on the bank's first matmul (arms the zero region),
                `last` puts stop=True on its final matmul. bias_base=None
                skips the bias matmul (continuation quarters)."""
                if bias_base is not None:
                    nc.tensor.matmul(dst, brow(bias_base), ones_sb,
                                     start=first, stop=False)
                    first = False
                for k in range(n_k):
                    nc.tensor.matmul(dst, wtile(w_sb, m_global, k, n_k),
                                     src[:, k * BC:(k + 1) * BC],
                                     start=(first and k == 0),
                                     stop=(last and k == n_k - 1))

            for t in range(nsteps):
                cs = lambda m: slice(m * BC, (m + 1) * BC)
                # NOTE: emission order defines the dependency DAG Tile sees.
                # Cross-engine producers are emitted immediately before their
                # first consumer; per-engine queue order is what matters for
                # overlap (PE: A B C D E ydec F G yenc / S: relu sr tanh sz
                # ycopies / V: t3 t4 d t6 h).
                e_t = temps.tile([128, 256], bf, tag="e")
                r_t = temps.tile([128, 256], bf, tag="r")
                z_t = temps.tile([128, 256], bf, tag="z")
                n_t = temps.tile([128, 256], bf, tag="n")
                t3 = temps.tile([128, 256], bf, tag="t3")
                t4 = temps.tile([128, 256], f32, tag="t4")
                d_t = temps.tile([128, 256], bf, tag="d")
                t6 = temps.tile([128, 256], bf, tag="t6")

                # A: e pre-activation (h-gated; step 0 uses emb @ x0)
                if t == 0:
                    x0 = encT_sb[:, lastblk]
                    for m in range(4):
                        gate_group(e_ps[:, cs(m)], BE0 + m, emb_sb, m, x0, 2,
                                   first=(m == 0), last=(m == 3))
                else:
                    for m in range(4):
                        gate_group(e_ps[:, cs(m)], BE + m, m1_sb, m, h_sb, 4,
                                   first=(m == 0), last=(m == 3))
                # B: whh-r halves of a_r groups (bank group stays open)
                for m in range(4):
                    gate_group(r_ps[:, cs(m)], BR + m, whh_sb, m, h_sb, 4,
                               first=(m == 0))
                # relu (scalar) - emitted before C which consumes e_t
                nc.scalar.activation(e_t, e_ps[:, 0:256], AF.Relu)
                # C: wih-r halves close the a_r bank group (relu-gated)
                for m in range(4):
                    gate_group(r_ps[:, cs(m)], None, wih_sb, m, e_t, 4,
                               last=(m == 3))
                # sigma_r (scalar)
                nc.scalar.activation(r_t, r_ps[:, 0:256], AF.Sigmoid)
                # D: hn region (h-gated)
                for m in range(4):
                    gate_group(hn_ps[:, cs(m)], BHN + m, whh_sb, 8 + m, h_sb, 4,
                               first=(m == 0), last=(m == 3))
                # t3 = hn * r  (DVE)
                nc.vector.tensor_tensor(t3, hn_ps[:, 0:256], r_t, OP.mult)
                # E: inn region (e-gated)
                for m in range(4):
                    gate_group(in_ps[:, cs(m)], BIN + m, wih_sb, 8 + m, e_t, 4,
                               first=(m == 0), last=(m == 3))
                # t4 = t3 + inn  (DVE)
                nc.vector.tensor_tensor(t4, t3, in_ps[:, 0:256], OP.add)
                # ydec: y token t-1 from h(t) (h-gated filler)
                if t > 0:
                    nc.tensor.matmul(y_ps[:, 64:128], ones_sb,
                                     ybias_sb[0:1, O:2 * O],
                                     start=True, stop=False)
                    for k in range(4):
                        nc.tensor.matmul(y_ps[:, 64:128],
                                         h_sb[:, k * BC:(k + 1) * BC],
                                         wyd_sb[:, k * O:(k + 1) * O],
                                         start=False, stop=(k == 3))
                # n = tanh(t4)  (scalar)
                nc.scalar.activation(n_t, t4, AF.Tanh)
                # F: whh-z halves (groups stay open)
                for m in range(4):
                    gate_group(z_ps[:, cs(m)], BZ + m, whh_sb, 4 + m, h_sb, 4,
                               first=(m == 0))
                # G: wih-z halves close a_z (e-gated)
                for m in range(4):
                    gate_group(z_ps[:, cs(m)], None, wih_sb, 4 + m, e_t, 4,
                               last=(m == 3))
                # sigma_z (scalar, after tanh in queue so tanh isn't blocked)
                nc.scalar.activation(z_t, z_ps[:, 0:256], AF.Sigmoid)
                # DVE tail: d = h - n; t6 = z*d; h' = n + t6 (in-place)
                nc.vector.tensor_tensor(d_t, h_sb, n_t, OP.subtract)
                nc.vector.tensor_tensor(t6, z_t, d_t, OP.mult)
                nc.vector.tensor_tensor(h_sb, n_t, t6, OP.add)
                # yenc: encoder token t projection (filler)
                if t < t_enc:
                    nc.tensor.matmul(y_ps[:, 0:64], ones_sb,
                                     ybias_sb[0:1, 0:O],
                                     start=True, stop=False)
                    for k in range(2):
                        nc.tensor.matmul(
                            y_ps[:, 0:64],
                            encT_sb[:, t * 128 + k * BC: t * 128 + (k + 1) * BC],
                            regw_sb[:, k * O:(k + 1) * O],
                            start=False, stop=(k == 1))

                # ---------------- y staging + DMA ----------------
                if t > 0:
                    y_dec = ytmp.tile([BC, O], f32, tag="ydec")
                    nc.scalar.activation(y_dec, y_ps[:, 64:128], AF.Identity)
                    nc.sync.dma_start(out=y_d[:, t_enc + t - 1, :], in_=y_dec)
                if t < t_enc:
                    y_enc = ytmp.tile([BC, O], f32, tag="yenc")
                    nc.scalar.activation(y_enc, y_ps[:, 0:64], AF.Identity)
                    nc.sync.dma_start(out=y_d[:, t, :], in_=y_enc)

            # final decoder token from h(nsteps)
            nc.tensor.matmul(y_ps[:, 64:128], ones_sb, ybias_sb[0:1, O:2 * O],
                             start=True, stop=False)
            for k in range(4):
                nc.tensor.matmul(y_ps[:, 64:128],
                                 h_sb[:, k * BC:(k + 1) * BC],
                                 wyd_sb[:, k * O:(k + 1) * O],
                                 start=False, stop=(k == 3))
            y_fin = ytmp.tile([BC, O], f32, tag="ydec")
            nc.scalar.activation(y_fin, y_ps[:, 64:128], AF.Identity)
            nc.sync.dma_start(out=y_d[:, t_enc + nsteps - 1, :], in_=y_fin)

            # leftover encoder tokens if nsteps < t_enc (smoke tests only)
            for t in range(nsteps, t_enc):
                nc.tensor.matmul(y_ps[:, 0:64], ones_sb, ybias_sb[0:1, 0:O],
                                 start=True, stop=False)
                for k in range(2):
                    nc.tensor.matmul(
                        y_ps[:, 0:64],
                        encT_sb[:, t * 128 + k * BC: t * 128 + (k + 1) * BC],
                        regw_sb[:, k * O:(k + 1) * O],
                        start=False, stop=(k == 1))
                y_enc = ytmp.tile([BC, O], f32, tag="yenc")
                nc.scalar.activation(y_enc, y_ps[:, 0:64], AF.Identity)
                nc.sync.dma_start(out=y_d[:, t, :], in_=y_enc)

    if lowering:
        nc.finalize()
    return nc


def prep_inputs(encoder_outputs, encoder_hidden, emb_W, emb_b, w_ih, w_hh,
                b_ih, b_hh, out_W, out_b, reg_W, reg_b, nsteps=PRED_LEN,
                t_enc=T_ENC):
    """Host-side packing. Returns per-core input dicts."""
    f32 = np.float32
    f64 = np.float64
    emb_W, emb_b, w_ih, w_hh, b_ih, b_hh, out_W, out_b, reg_W, reg_b = (
        np.asarray(a, f32) for a in
        (emb_W, emb_b, w_ih, w_hh, b_ih, b_hh, out_W, out_b, reg_W, reg_b))

    # fused weights (x = h@out_W.T + out_b is linear in h)
    M1 = (out_W.T.astype(f64) @ emb_W.T.astype(f64)).astype(f32)     # [H, H]
    c_e = (emb_b + out_b @ emb_W.T).astype(f32)                      # [H]
    Wyd = (out_W.T.astype(f64) @ reg_W.T.astype(f64)).astype(f32)    # [H, O]
    c_yd = (out_b @ reg_W.T + reg_b).astype(f32)                     # [O]

    def rows4(v):      # [512] -> [4, 128] rows flattened
        return v.reshape(4, 128)

    bias_pack = np.concatenate([
        rows4(c_e), rows4(emb_b),
        rows4(b_ih[:H] + b_hh[:H]),
        rows4(b_ih[H:2 * H] + b_hh[H:2 * H]),
        rows4(b_hh[2 * H:]), rows4(b_ih[2 * H:]),
    ], axis=0).reshape(1, 24 * 128)

    shared = {
        "m1T": _pack_tiles(M1, 4, 4),
        "wihT": _pack_tiles(w_ih.T, 4, 12),
        "whhT": _pack_tiles(w_hh.T, 4, 12),
        "embT": _pack_tiles(emb_W.T, 2, 4),
        "wydT": np.ascontiguousarray(
            Wyd.reshape(4, 128, O).transpose(1, 0, 2).reshape(128, 4 * O)
            .astype(bf16)),
        "regwT": np.ascontiguousarray(
            reg_W.T.reshape(2, 128, O).transpose(1, 0, 2).reshape(128, 2 * O)
            .astype(bf16)),
        "biasP": np.ascontiguousarray(bias_pack.astype(bf16)),
        "ones1": np.ones((1, BC), bf16),
        "ybias": np.ascontiguousarray(
            np.concatenate([reg_b, c_yd]).reshape(1, 2 * O).astype(bf16)),
    }

    enc = np.asarray(encoder_outputs, f32)[:, :t_enc, :]
    h0 = np.asarray(encoder_hidden, f32)[0]
    in_maps = []
    for i in range(NCORES):
        sl = slice(i * BC, (i + 1) * BC)
        enc_i = enc[sl].astype(bf16)              # [BC, t_enc, E]
        encT = (enc_i.reshape(BC, t_enc, 2, 128).transpose(3, 1, 2, 0)
                .reshape(128, t_enc * 128))
        m = dict(shared)
        m["encT"] = np.ascontiguousarray(encT)
        m["h0T"] = _feat_major(h0[sl], 4).astype(bf16)
        in_maps.append(m)
    return in_maps


def kernel(encoder_outputs, encoder_hidden, emb_W, emb_b, w_ih, w_hh,
           b_ih, b_hh, out_W, out_b, reg_W, reg_b):
    from concourse.bass_utils import run_bass_kernel_spmd

    nc = build_program()
    in_maps = prep_inputs(encoder_outputs, encoder_hidden, emb_W, emb_b,
                          w_ih, w_hh, b_ih, b_hh, out_W, out_b, reg_W, reg_b)
    res = run_bass_kernel_spmd(nc, in_maps, core_ids=list(range(NCORES)))
    out = np.empty((B, T_ALL, O), np.float32)
    for i in range(NCORES):
        out[i * BC:(i + 1) * BC] = res.results[i]["y"]
    return out


# revision 25
# speedup vs baseline: 1.9605x; 1.9605x over previous
"""Trainium2 Bass kernel for nn_DecoderRNN (GRU decoder, 140 sequential steps).

Strategy (data-parallel, per sharding hint):
  - B=512 sharded 8 ways -> 64 batch rows per core; weights replicated.
  - Feature-major on-chip layout: a [F, B] tensor is F/128 chunks of
    [128 partitions, 64 batch] side by side in the free dim.
  - Weight fusion: the fed-back x = h @ out_W.T + out_b is linear in h, so
    it is folded into the next step's embedding (M1 = out_W.T @ emb_W.T)
    and into the decoder output projection (Wyd = out_W.T @ reg_W.T).
    x is never materialized; one full serial stage per step disappears.
  - Gate math runs as fused [128, 256] ops (1 relu, 2 sigmoid, 1 tanh on
    the scalar engine; 5 tensor_tensor on DVE) instead of per-chunk
    [128, 64] ops - per-op fixed overhead dominates at this size.
  - All gate biases are folded into PSUM by tiny K=1 matmuls (lhsT = bias
    row [1,128], rhs = ones [1,64]) at the start of each accumulation
    group, so the fused activations need no per-chunk bias APs.
  - PE phase order per step keeps independent matmul work queued behind
    every cross-engine dependency so the PE (and its DVFS clock) stays fed.
  - Everything bf16 except PSUM (fp32), tanh pre-activation (fp32) and the
    f32 output; validated vs f32 reference at rel err ~2e-3.
"""

import numpy as np
import ml_dtypes

B, T_ENC, E, H, O, PRED_LEN = 512, 140, 256, 512, 64, 140
NCORES = 8
BC = B // NCORES           # 64 batch rows per core
T_ALL = T_ENC + PRED_LEN   # 280

bf16 = ml_dtypes.bfloat16


def _pack_tiles(wT, n_k, n_m):
    """Pack a [K, M] (pre-transposed) weight into [128, n_m*n_k*128] bf16:
    tile (mi, k) at cols (mi*n_k + k)*128."""
    K, M = wT.shape
    assert K == n_k * 128 and M == n_m * 128
    t = wT.reshape(n_k, 128, n_m, 128).transpose(2, 0, 1, 3)  # [mc, kc, 128, 128]
    t = t.transpose(2, 0, 1, 3).reshape(128, -1)
    return np.ascontiguousarray(t.astype(bf16))


def _feat_major(x, n_chunks):
    """[B, F] -> [128, n_chunks*B] feature-major chunk layout."""
    b, f = x.shape
    assert f == n_chunks * 128
    t = x.reshape(b, n_chunks, 128).transpose(2, 1, 0).reshape(128, n_chunks * b)
    return np.ascontiguousarray(t)


def build_program(nsteps=PRED_LEN, t_enc=T_ENC, lowering=True):
    """Build the Bass program (per-core SPMD). Returns nc."""
    import concourse.bass as bass
    import concourse.tile as tile
    from concourse import bacc, mybir

    AF = mybir.ActivationFunctionType
    OP = mybir.AluOpType
    f32 = mybir.dt.float32
    bf = mybir.dt.bfloat16

    t_all = t_enc + nsteps

    if lowering:
        nc = bacc.Bacc("TRN2", target_bir_lowering=True, debug=False)
    else:
        nc = bass.Bass("TRN2", target_bir_lowering=False, debug=False)

    # ---- DRAM I/O ----
    encT_d = nc.dram_tensor("encT", [128, t_enc * 128], bf, kind="ExternalInput").ap()
    h0_d = nc.dram_tensor("h0T", [128, 4 * BC], bf, kind="ExternalInput").ap()
    m1_d = nc.dram_tensor("m1T", [128, 16 * 128], bf, kind="ExternalInput").ap()
    wih_d = nc.dram_tensor("wihT", [128, 48 * 128], bf, kind="ExternalInput").ap()
    whh_d = nc.dram_tensor("whhT", [128, 48 * 128], bf, kind="ExternalInput").ap()
    emb_d = nc.dram_tensor("embT", [128, 8 * 128], bf, kind="ExternalInput").ap()
    wyd_d = nc.dram_tensor("wydT", [128, 4 * O], bf, kind="ExternalInput").ap()
    regw_d = nc.dram_tensor("regwT", [128, 2 * O], bf, kind="ExternalInput").ap()
    # bias weight tiles [128,128], single-row content (row 0 = bias chunk,
    # rest 0) so a regular K=128 matmul against a constant ones chunk adds
    # the bias exactly.  Tile j at cols j*128; region order:
    # e(4) e0(4) r(4) z(4) hn(4) inn(4)
    bias_d = nc.dram_tensor("biasT", [128, 24 * 128], bf, kind="ExternalInput").ap()
    # y-bias rhs tiles [128, 2*O]: row 0 of block 0 = reg_b, block 1 = c_yd
    ybias_d = nc.dram_tensor("ybT", [128, 2 * O], bf, kind="ExternalInput").ap()
    y_d = nc.dram_tensor("y", [BC, t_all, O], f32, kind="ExternalOutput").ap()

    with tile.TileContext(nc) as tc:
        import contextlib
        with contextlib.ExitStack() as ctx:
            consts = ctx.enter_context(tc.tile_pool(name="consts", bufs=1))
            temps = ctx.enter_context(tc.tile_pool(name="temps", bufs=2))
            ytmp = ctx.enter_context(tc.tile_pool(name="ytmp", bufs=3))
            psum = ctx.enter_context(tc.tile_pool(name="psum", bufs=1, space="PSUM"))

            # ---- ACT table warmup (see baseline note: pin the table load
            # to dependency-light dummy ops) ----
            wt = consts.tile([128, 10], f32, tag="wtbl", name="wtbl")
            nc.vector.memset(wt[:, 0:5], 0.0)
            nc.scalar.activation(wt[:, 5:6], wt[:, 0:1], AF.Relu)
            nc.scalar.activation(wt[:, 6:7], wt[:, 1:2], AF.Sigmoid)
            nc.scalar.activation(wt[:, 7:8], wt[:, 2:3], AF.Tanh)
            nc.scalar.activation(wt[:, 8:9], wt[:, 3:4], AF.Identity)

            # ---- load constants into SBUF ----
            m1_sb = consts.tile([128, 16 * 128], bf, tag="m1")
            wih_sb = consts.tile([128, 48 * 128], bf, tag="wih")
            whh_sb = consts.tile([128, 48 * 128], bf, tag="whh")
            emb_sb = consts.tile([128, 8 * 128], bf, tag="emb")
            wyd_sb = consts.tile([128, 4 * O], bf, tag="wyd")
            regw_sb = consts.tile([128, 2 * O], bf, tag="regw")
            bias_sb = consts.tile([128, 24 * 128], bf, tag="biasT")
            ybias_sb = consts.tile([128, 2 * O], bf, tag="ybT")
            encT_sb = consts.tile([128, t_enc * 128], bf, tag="encT")

            nc.sync.dma_start(out=emb_sb, in_=emb_d)
            nc.sync.dma_start(out=m1_sb, in_=m1_d)
            nc.sync.dma_start(out=whh_sb, in_=whh_d)
            nc.sync.dma_start(out=wih_sb, in_=wih_d)
            nc.sync.dma_start(out=wyd_sb, in_=wyd_d)
            nc.sync.dma_start(out=regw_sb, in_=regw_d)
            nc.sync.dma_start(out=bias_sb, in_=bias_d)
            nc.sync.dma_start(out=ybias_sb, in_=ybias_d)
            # x0 block (last encoder token) first so step 0 can start early
            lastblk = slice((t_enc - 1) * 128, t_enc * 128)
            nc.sync.dma_start(out=encT_sb[:, lastblk], in_=encT_d[:, lastblk])
            nsplit = 4
            per = (t_enc - 1) // nsplit + 1
            for i in range(nsplit):
                lo, hi = i * per, min((i + 1) * per, t_enc - 1)
                if lo >= hi:
                    continue
                nc.sync.dma_start(out=encT_sb[:, lo * 128:hi * 128],
                                  in_=encT_d[:, lo * 128:hi * 128])

            # ---- persistent state: h + a constant ones chunk at [4BC:5BC]
            # (the rhs stream for every bias-tile matmul) ----
            h_sb = consts.tile([128, 5 * BC], bf, tag="h", name="h")
            nc.sync.dma_start(out=h_sb[:, 0:4 * BC], in_=h0_d)
            nc.vector.memset(h_sb[:, 4 * BC:5 * BC], 1.0)
            h_ones = h_sb[:, 4 * BC:5 * BC]

            # ---- persistent PSUM regions (one bank each: a matmul with
            # start=True arms a whole 2KB zero region, and only one
            # accumulation group may be open per bank) ----
            e_ps = psum.tile([128, 512], f32, tag="eps")       # e pre-act
            r_ps = psum.tile([128, 512], f32, tag="rps")       # a_r
            z_ps = psum.tile([128, 512], f32, tag="zps")       # a_z
            hn_ps = psum.tile([128, 512], f32, tag="hnps")     # hn
            in_ps = psum.tile([128, 512], f32, tag="inps")     # inn
            # y banks ping-pong by step parity so step t's matmuls never
            # wait on step t-1's psum->sbuf copies
            y_pp = [psum.tile([BC, 512], f32, tag=f"yps{s}", name=f"yps{s}")
                    for s in range(2)]

            def wtile(sb, mi, k, n_k):
                j = (mi * n_k + k) * 128
                return sb[:, j:j + 128]

            def btile(idx):
                return bias_sb[:, idx * 128:(idx + 1) * 128]

            # bias tile base indices in bias_sb
            BE, BE0, BR, BZ, BHN, BIN = 0, 4, 8, 12, 16, 20

            def gate_group(dst, bias_base, w_sb, m_global, src, n_k,
                           first=False, last=False):
                """m-quarter of a bank-level accumulation group. `first` puts
                start=True on the bank's first matmul (arms the zero region),
                `last` puts stop=True on its final matmul. bias_base=None
                skips the bias matmul (continuation quarters)."""
                if bias_base is not None:
                    nc.tensor.matmul(dst, btile(bias_base), h_ones,
                                     start=first, stop=False)
                    first = False
                for k in range(n_k):
                    nc.tensor.matmul(dst, wtile(w_sb, m_global, k, n_k),
                                     src[:, k * BC:(k + 1) * BC],
                                     start=(first and k == 0),
                                     stop=(last and k == n_k - 1))

            for t in range(nsteps):
                cs = lambda m: slice(m * BC, (m + 1) * BC)
                # NOTE: emission order defines the dependency DAG Tile sees.
                # Cross-engine producers are emitted immediately before their
                # first consumer; per-engine queue order is what matters for
                # overlap (PE: A B C D E ydec F G yenc / S: relu sr tanh sz
                # ycopies / V: t3 t4 d t6 h).
                e_t = temps.tile([128, 256], bf, tag="e")
                r_t = temps.tile([128, 256], bf, tag="r")
                z_t = temps.tile([128, 256], bf, tag="z")
                n_t = temps.tile([128, 256], bf, tag="n")
                t3 = temps.tile([128, 256], bf, tag="t3")
                t4 = temps.tile([128, 256], f32, tag="t4")
                d_t = temps.tile([128, 256], bf, tag="d")
                t6 = temps.tile([128, 256], bf, tag="t6")

                # A: e pre-activation (h-gated; step 0 uses emb @ x0)
                if t == 0:
                    x0 = encT_sb[:, lastblk]
                    for m in range(4):
                        gate_group(e_ps[:, cs(m)], BE0 + m, emb_sb, m, x0, 2,
                                   first=(m == 0), last=(m == 3))
                else:
                    for m in range(4):
                        gate_group(e_ps[:, cs(m)], BE + m, m1_sb, m, h_sb, 4,
                                   first=(m == 0), last=(m == 3))
                # B: whh-r halves of a_r groups (bank group stays open)
                for m in range(4):
                    gate_group(r_ps[:, cs(m)], BR + m, whh_sb, m, h_sb, 4,
                               first=(m == 0))
                # relu (scalar) - emitted before C which consumes e_t
                nc.scalar.activation(e_t, e_ps[:, 0:256], AF.Relu)
                # C: wih-r halves close the a_r bank group (relu-gated)
                for m in range(4):
                    gate_group(r_ps[:, cs(m)], None, wih_sb, m, e_t, 4,
                               last=(m == 3))
                # sigma_r (scalar)
                nc.scalar.activation(r_t, r_ps[:, 0:256], AF.Sigmoid)
                # D: hn region (h-gated)
                for m in range(4):
                    gate_group(hn_ps[:, cs(m)], BHN + m, whh_sb, 8 + m, h_sb, 4,
                               first=(m == 0), last=(m == 3))
                # t3 = hn * r  (DVE)
                nc.vector.tensor_tensor(t3, hn_ps[:, 0:256], r_t, OP.mult)
                # E: inn region (e-gated)
                for m in range(4):
                    gate_group(in_ps[:, cs(m)], BIN + m, wih_sb, 8 + m, e_t, 4,
                               first=(m == 0), last=(m == 3))
                # t4 = t3 + inn  (DVE)
                nc.vector.tensor_tensor(t4, t3, in_ps[:, 0:256], OP.add)
                # ydec: y token t-1 from h(t) (h-gated filler)
                y_ps = y_pp[t % 2]
                if t > 0:
                    nc.tensor.matmul(y_ps[:, 64:128], h_ones,
                                     ybias_sb[:, O:2 * O],
                                     start=True, stop=False)
                    for k in range(4):
                        nc.tensor.matmul(y_ps[:, 64:128],
                                         h_sb[:, k * BC:(k + 1) * BC],
                                         wyd_sb[:, k * O:(k + 1) * O],
                                         start=False, stop=(k == 3))
                # n = tanh(t4)  (scalar)
                nc.scalar.activation(n_t, t4, AF.Tanh)
                # F: whh-z halves (groups stay open)
                for m in range(4):
                    gate_group(z_ps[:, cs(m)], BZ + m, whh_sb, 4 + m, h_sb, 4,
                               first=(m == 0))
                # G: wih-z halves close a_z (e-gated)
                for m in range(4):
                    gate_group(z_ps[:, cs(m)], None, wih_sb, 4 + m, e_t, 4,
                               last=(m == 3))
                # sigma_z (scalar, after tanh in queue so tanh isn't blocked)
                nc.scalar.activation(z_t, z_ps[:, 0:256], AF.Sigmoid)
                # DVE tail: d = h - n; t6 = z*d; h' = n + t6 (in-place)
                nc.vector.tensor_tensor(d_t, h_sb[:, 0:256], n_t, OP.subtract)
                nc.vector.tensor_tensor(t6, z_t, d_t, OP.mult)
                nc.vector.tensor_tensor(h_sb[:, 0:256], n_t, t6, OP.add)
                # yenc: encoder token t projection (filler)
                if t < t_enc:
                    nc.tensor.matmul(y_ps[:, 0:64], h_ones,
                                     ybias_sb[:, 0:O],
                                     start=True, stop=False)
                    for k in range(2):
                        nc.tensor.matmul(
                            y_ps[:, 0:64],
                            encT_sb[:, t * 128 + k * BC: t * 128 + (k + 1) * BC],
                            regw_sb[:, k * O:(k + 1) * O],
                            start=False, stop=(k == 1))

                # ---------------- y staging + DMA ----------------
                if t > 0:
                    y_dec = ytmp.tile([BC, O], f32, tag="ydec")
                    nc.scalar.activation(y_dec, y_ps[:, 64:128], AF.Identity)
                    nc.sync.dma_start(out=y_d[:, t_enc + t - 1, :], in_=y_dec)
                if t < t_enc:
                    y_enc = ytmp.tile([BC, O], f32, tag="yenc")
                    nc.scalar.activation(y_enc, y_ps[:, 0:64], AF.Identity)
                    nc.sync.dma_start(out=y_d[:, t, :], in_=y_enc)

            # final decoder token from h(nsteps)
            y_ps = y_pp[nsteps % 2]
            nc.tensor.matmul(y_ps[:, 64:128], h_ones, ybias_sb[:, O:2 * O],
                             start=True, stop=False)
            for k in range(4):
                nc.tensor.matmul(y_ps[:, 64:128],
                                 h_sb[:, k * BC:(k + 1) * BC],
                                 wyd_sb[:, k * O:(k + 1) * O],
                                 start=False, stop=(k == 3))
            y_fin = ytmp.tile([BC, O], f32, tag="ydec")
            nc.scalar.activation(y_fin, y_ps[:, 64:128], AF.Identity)
            nc.sync.dma_start(out=y_d[:, t_enc + nsteps - 1, :], in_=y_fin)

            # leftover encoder tokens if nsteps < t_enc (smoke tests only)
            for t in range(nsteps, t_enc):
                y_ps = y_pp[t % 2]
                nc.tensor.matmul(y_ps[:, 0:64], h_ones, ybias_sb[:, 0:O],
                                 start=True, stop=False)
                for k in range(2):
                    nc.tensor.matmul(
                        y_ps[:, 0:64],
                        encT_sb[:, t * 128 + k * BC: t * 128 + (k + 1) * BC],
                        regw_sb[:, k * O:(k + 1) * O],
                        start=False, stop=(k == 1))
                y_enc = ytmp.tile([BC, O], f32, tag="yenc")
                nc.scalar.activation(y_enc, y_ps[:, 0:64], AF.Identity)
                nc.sync.dma_start(out=y_d[:, t, :], in_=y_enc)

    if lowering:
        nc.finalize()
    return nc


def prep_inputs(encoder_outputs, encoder_hidden, emb_W, emb_b, w_ih, w_hh,
                b_ih, b_hh, out_W, out_b, reg_W, reg_b, nsteps=PRED_LEN,
                t_enc=T_ENC):
    """Host-side packing. Returns per-core input dicts."""
    f32 = np.float32
    f64 = np.float64
    emb_W, emb_b, w_ih, w_hh, b_ih, b_hh, out_W, out_b, reg_W, reg_b = (
        np.asarray(a, f32) for a in
        (emb_W, emb_b, w_ih, w_hh, b_ih, b_hh, out_W, out_b, reg_W, reg_b))

    # fused weights (x = h@out_W.T + out_b is linear in h)
    M1 = (out_W.T.astype(f64) @ emb_W.T.astype(f64)).astype(f32)     # [H, H]
    c_e = (emb_b + out_b @ emb_W.T).astype(f32)                      # [H]
    Wyd = (out_W.T.astype(f64) @ reg_W.T.astype(f64)).astype(f32)    # [H, O]
    c_yd = (out_b @ reg_W.T + reg_b).astype(f32)                     # [O]

    # bias weight tiles: [128, 24*128], tile j all-zero except row 0 which
    # holds the j-th bias chunk (region order e, e0, r, z, hn, inn)
    bias_rows = np.concatenate([
        c_e, emb_b,
        b_ih[:H] + b_hh[:H],
        b_ih[H:2 * H] + b_hh[H:2 * H],
        b_hh[2 * H:], b_ih[2 * H:],
    ]).reshape(24, 128)
    bias_pack = np.zeros((128, 24 * 128), f32)
    for j in range(24):
        bias_pack[0, j * 128:(j + 1) * 128] = bias_rows[j]

    yb_pack = np.zeros((128, 2 * O), f32)
    yb_pack[0, 0:O] = reg_b
    yb_pack[0, O:2 * O] = c_yd

    shared = {
        "m1T": _pack_tiles(M1, 4, 4),
        "wihT": _pack_tiles(w_ih.T, 4, 12),
        "whhT": _pack_tiles(w_hh.T, 4, 12),
        "embT": _pack_tiles(emb_W.T, 2, 4),
        "wydT": np.ascontiguousarray(
            Wyd.reshape(4, 128, O).transpose(1, 0, 2).reshape(128, 4 * O)
            .astype(bf16)),
        "regwT": np.ascontiguousarray(
            reg_W.T.reshape(2, 128, O).transpose(1, 0, 2).reshape(128, 2 * O)
            .astype(bf16)),
        "biasT": np.ascontiguousarray(bias_pack.astype(bf16)),
        "ybT": np.ascontiguousarray(yb_pack.astype(bf16)),
    }

    enc = np.asarray(encoder_outputs, f32)[:, :t_enc, :]
    h0 = np.asarray(encoder_hidden, f32)[0]
    in_maps = []
    for i in range(NCORES):
        sl = slice(i * BC, (i + 1) * BC)
        enc_i = enc[sl].astype(bf16)              # [BC, t_enc, E]
        encT = (enc_i.reshape(BC, t_enc, 2, 128).transpose(3, 1, 2, 0)
                .reshape(128, t_enc * 128))
        m = dict(shared)
        m["encT"] = np.ascontiguousarray(encT)
        m["h0T"] = _feat_major(h0[sl], 4).astype(bf16)
        in_maps.append(m)
    return in_maps


def kernel(encoder_outputs, encoder_hidden, emb_W, emb_b, w_ih, w_hh,
           b_ih, b_hh, out_W, out_b, reg_W, reg_b):
    from concourse.bass_utils import run_bass_kernel_spmd

    nc = build_program()
    in_maps = prep_inputs(encoder_outputs, encoder_hidden, emb_W, emb_b,
                          w_ih, w_hh, b_ih, b_hh, out_W, out_b, reg_W, reg_b)
    res = run_bass_kernel_spmd(nc, in_maps, core_ids=list(range(NCORES)))
    out = np.empty((B, T_ALL, O), np.float32)
    for i in range(NCORES):
        out[i * BC:(i + 1) * BC] = res.results[i]["y"]
    return out
